# revision 18
# baseline (speedup 1.0000x reference)
"""BiLSTM-CRF (Viterbi decode) Trainium2 kernel.

Three SPMD launches (collectives are unavailable in this environment, so the
tiny reduction/reshuffle steps between phases happen on the host):

  Launch A (8 cores): the 50000x300 embedding table is sharded row-wise
    across cores (per the sharding hint). Each core indirect-DMA-gathers its
    shard's rows for all L positions (out-of-shard tokens hit a zero pad
    row) and writes a partial X [L, E]. Host sums the 8 partials.
  Launch B (2 cores): core 0 runs the forward LSTM, core 1 the backward one
    (same program - core 1 just receives X reversed and its own weights).
    Each core computes Z^T = W_ih @ X^T + b on the PE, then the sequential
    recurrence with h in partition layout [128, 2] (16 small stationary
    matmuls + DVE/ACT gate ops per step), then the output projection
    hproj = h_seq @ W_out_half^T [L, T]. Host adds the two halves.
  Launch C (1 core): Viterbi. Forward pass is 5 DVE-only ops per step
    (tensor_scalar add with per-partition fv, 32x32 stream transpose, max,
    max_index, feat add); backpointers stream to DRAM; an all-DVE backtrace
    via one-hot dot products reconstructs the path exactly.

kernel(**inputs) takes the full unsharded reference inputs and returns
(score, path) like the reference.
"""

import numpy as np

import concourse.bass as bass
import concourse.mybir as mybir
import concourse.tile as tile
from concourse import bacc
from concourse.bass import ds, ts
from concourse.masks import make_identity

F32 = mybir.dt.float32
I32 = mybir.dt.int32
U16 = mybir.dt.uint16
AF = mybir.ActivationFunctionType
OP = mybir.AluOpType

V_FULL, L_FULL, E_FULL = 50000, 4096, 300
H2, T = 256, 16
START, STOP = 14, 15
NEG = -10000.0
CH = 128


def _ceil_to(x, m):
    return (x + m - 1) // m * m


class Cfg:
    def __init__(self, V=V_FULL, L=L_FULL, E=E_FULL, n_gather=8):
        self.V, self.L, self.E, self.n_gather = V, L, E, n_gather
        self.EP = _ceil_to(E, 16)
        self.KT = [128] * (self.EP // 128) + ([self.EP % 128] if self.EP % 128 else [])
        self.NB = L // CH
        self.shard = _ceil_to((V + n_gather - 1) // n_gather, 8)
        self.shard_rows = self.shard + 8
        self.NJ = 4 * H2 // 128
        self.NKT = H2 // 128

    @property
    def key(self):
        return (self.V, self.L, self.E, self.n_gather)


# ----------------------------------------------------------------------------
# Launch A: sharded embedding gather
# ----------------------------------------------------------------------------

def build_gather(cfg: Cfg):
    nc = bacc.Bacc("TRN2", target_bir_lowering=False, debug=False,
                   num_devices=cfg.n_gather)
    L, EP, NB = cfg.L, cfg.EP, cfg.NB
    emb_in = nc.dram_tensor("embshard", [cfg.shard_rows, EP], F32,
                            kind="ExternalInput")
    tok_in = nc.dram_tensor("tokfold", [128, NB], I32, kind="ExternalInput")
    xp_out = nc.dram_tensor("xpart", [L, EP], F32, kind="ExternalOutput")

    with tile.TileContext(nc) as tc:
        with tc.tile_pool(name="sb", bufs=1) as sb, \
             tc.tile_pool(name="p1", bufs=4) as p1:
            tok_sb = sb.tile([128, NB], I32, name="tok_sb")
            nc.sync.dma_start(tok_sb[:], tok_in[:])
            for b in range(NB):
                gx = p1.tile([128, EP], F32, name="gx", tag="gx")
                nc.gpsimd.indirect_dma_start(
                    out=gx[:], out_offset=None, in_=emb_in[:],
                    in_offset=bass.IndirectOffsetOnAxis(
                        ap=tok_sb[:, b:b + 1], axis=0),
                )
                nc.sync.dma_start(xp_out[ts(b, 128), :], gx[:])
    nc.compile()
    return nc


# ----------------------------------------------------------------------------
# Launch B: Z precompute + LSTM recurrence + projection (2 cores: fwd/bwd)
# ----------------------------------------------------------------------------

def build_lstm(cfg: Cfg):
    nc = bacc.Bacc("TRN2", target_bir_lowering=False, debug=False,
                   num_devices=2)
    L, EP, NB, NJ, NKT = cfg.L, cfg.EP, cfg.NB, cfg.NJ, cfg.NKT
    H4 = 4 * H2

    x_in = nc.dram_tensor("xin", [L, EP], F32, kind="ExternalInput")
    wih_in = nc.dram_tensor("wihT", [128, len(cfg.KT) * H4], F32,
                            kind="ExternalInput")
    bfold_in = nc.dram_tensor("bfold", [128, NJ], F32, kind="ExternalInput")
    whh_in = nc.dram_tensor("whhpack", [128, NJ * NKT * 128], F32,
                            kind="ExternalInput")
    hc0_in = nc.dram_tensor("hc0fold", [128, 4], F32, kind="ExternalInput")
    wo_in = nc.dram_tensor("woT", [128, NKT * T], F32, kind="ExternalInput")
    hp_out = nc.dram_tensor("hproj", [L, T], F32, kind="ExternalOutput")

    with tile.TileContext(nc) as tc:
        with tc.tile_pool(name="dram", bufs=1, space="DRAM") as dram:
            zt_dram = dram.tile([H4, L], F32, name="zt_dram")
            with tc.tile_pool(name="const", bufs=1) as const:
                ident = const.tile([128, 128], F32, name="ident")
                make_identity(nc, ident[:])

                # ===== X^T via PE transposes =====
                xT = const.tile([128, len(cfg.KT) * L], F32, name="xT")
                with tc.tile_pool(name="p1", bufs=3) as p1, \
                     tc.tile_pool(name="p1ps", bufs=2, space="PSUM") as p1ps:
                    for b in range(NB):
                        xo = p1.tile([128, EP], F32, name="xo", tag="xo")
                        nc.sync.dma_start(xo[:], x_in[ts(b, 128), :])
                        for kt, ksz in enumerate(cfg.KT):
                            pt = p1ps.tile([128, 128], F32, name="pt", tag="pt")
                            nc.tensor.transpose(
                                pt[:ksz, :], xo[:, kt * 128:kt * 128 + ksz],
                                ident[:])
                            nc.vector.tensor_copy(
                                xT[:ksz, kt * L + b * 128:kt * L + (b + 1) * 128],
                                pt[:ksz, :])

                    # ===== Z^T = W_ih @ X^T + b =====
                    wih_sb = p1.tile([128, len(cfg.KT) * H4], F32,
                                     name="wih_sb", tag="wih")
                    nc.sync.dma_start(wih_sb[:], wih_in[:])
                    bf_sb = const.tile([128, NJ], F32, name="bf_sb")
                    nc.sync.dma_start(bf_sb[:], bfold_in[:])
                    FW = min(512, L)
                    NF = L // FW
                    for fc in range(NF):
                        for mt in range(NJ):
                            pz = p1ps.tile([128, FW], F32, name="pz", tag="pz")
                            for kt, ksz in enumerate(cfg.KT):
                                nc.tensor.matmul(
                                    pz[:],
                                    wih_sb[:ksz, kt * H4 + mt * 128:
                                           kt * H4 + (mt + 1) * 128],
                                    xT[:ksz, kt * L + fc * FW:
                                       kt * L + fc * FW + FW],
                                    start=(kt == 0),
                                    stop=(kt == len(cfg.KT) - 1),
                                )
                            zsb = p1.tile([128, FW], F32, name="zsb", tag="zsb")
                            nc.vector.tensor_scalar(
                                zsb[:], pz[:], bf_sb[:, mt:mt + 1], None,
                                OP.add)
                            nc.sync.dma_start(
                                zt_dram[ts(mt, 128), fc * FW:fc * FW + FW],
                                zsb[:])

                # ===== LSTM recurrence =====
                whh_sb = const.tile([128, NJ * NKT * 128], F32, name="whh_sb")
                nc.sync.dma_start(whh_sb[:], whh_in[:])
                hc0_sb = const.tile([128, 4], F32, name="hc0_sb")
                nc.sync.dma_start(hc0_sb[:], hc0_in[:])
                hseq = const.tile([128, 2 * L], F32, name="hseq")
                c_a = const.tile([128, 2], F32, name="c_a")
                c_b = const.tile([128, 2], F32, name="c_b")
                nc.vector.tensor_copy(c_a[:], hc0_sb[:, 2:4])

                with tc.tile_pool(name="p2", bufs=2) as p2, \
                     tc.tile_pool(name="p2ps", bufs=2, space="PSUM") as p2ps:
                    for c in range(NB):
                        zc = p2.tile([128, NJ * 128], F32, name="zc", tag="zc")
                        for j in range(NJ):
                            nc.sync.dma_start(
                                zc[:, ts(j, 128)],
                                zt_dram[ts(j, 128), ts(c, 128)])
                        for tp in range(128):
                            g = c * 128 + tp
                            if g == 0:
                                h_prev = hc0_sb[:, 0:2]
                            else:
                                h_prev = hseq[:, 2 * (g - 1):2 * g]
                            c_prev = (c_a if g % 2 == 0 else c_b)
                            c_next = (c_b if g % 2 == 0 else c_a)
                            pz2 = p2ps.tile([128, NJ], F32, name="pz2",
                                            tag="pz2")
                            for j in range(NJ):
                                for kt in range(NKT):
                                    nc.tensor.matmul(
                                        pz2[:, j:j + 1],
                                        whh_sb[:, (j * NKT + kt) * 128:
                                               (j * NKT + kt + 1) * 128],
                                        h_prev[:, kt:kt + 1],
                                        start=(kt == 0), stop=(kt == NKT - 1),
                                    )
                            zs = p2.tile([128, NJ], F32, name="zs", tag="zs")
                            nc.vector.tensor_tensor(
                                out=zs[:], in0=pz2[:],
                                in1=zc[:, tp:tp + (NJ - 1) * 128 + 1:128],
                                op=OP.add)
                            gt = p2.tile([128, NJ], F32, name="gt", tag="gt")
                            nc.scalar.activation(gt[:, 0:6], zs[:, 0:6],
                                                 AF.Sigmoid)
                            nc.scalar.activation(gt[:, 6:8], zs[:, 6:8],
                                                 AF.Tanh)
                            ig = p2.tile([128, 2], F32, name="ig", tag="ig")
                            nc.vector.tensor_tensor(
                                out=ig[:], in0=gt[:, 0:2], in1=gt[:, 6:8],
                                op=OP.mult)
                            fc2 = p2.tile([128, 2], F32, name="fc2", tag="fc2")
                            nc.vector.tensor_tensor(
                                out=fc2[:], in0=gt[:, 2:4], in1=c_prev[:],
                                op=OP.mult)
                            nc.vector.tensor_tensor(
                                out=c_next[:], in0=fc2[:], in1=ig[:],
                                op=OP.add)
                            tc_t = p2.tile([128, 2], F32, name="tc_t",
                                           tag="tc_t")
                            nc.scalar.activation(tc_t[:], c_next[:], AF.Tanh)
                            nc.vector.tensor_tensor(
                                out=hseq[:, 2 * g:2 * g + 2],
                                in0=gt[:, 4:6], in1=tc_t[:], op=OP.mult)

                    # ===== projection =====
                    wo_sb = p2.tile([128, NKT * T], F32, name="wo_sb", tag="wo")
                    nc.sync.dma_start(wo_sb[:], wo_in[:])
                    for mt in range(NB):
                        pp = p2ps.tile([128, T], F32, name="pp", tag="pp")
                        for kt in range(NKT):
                            st = 2 * 128 * mt + kt
                            nc.tensor.matmul(
                                pp[:], hseq[:, st:st + 255:2],
                                wo_sb[:, kt * T:(kt + 1) * T],
                                start=(kt == 0), stop=(kt == NKT - 1))
                        hp = p2.tile([128, T], F32, name="hp", tag="hp")
                        nc.vector.tensor_copy(hp[:], pp[:])
                        nc.sync.dma_start(hp_out[ts(mt, 128), :], hp[:])
    nc.compile()
    return nc


# ----------------------------------------------------------------------------
# Launch C: Viterbi decode (1 core)
# ----------------------------------------------------------------------------

def build_viterbi(cfg: Cfg):
    nc = bacc.Bacc("TRN2", target_bir_lowering=False, debug=False,
                   num_devices=1)
    L, NB = cfg.L, cfg.NB

    feats_in = nc.dram_tensor("feats", [L, T], F32, kind="ExternalInput")
    transT_in = nc.dram_tensor("transT", [T, T], F32, kind="ExternalInput")
    tstop_in = nc.dram_tensor("tstopcol", [T, 1], F32, kind="ExternalInput")
    bout_in = nc.dram_tensor("boutcol", [T, 1], F32, kind="ExternalInput")
    fvinit_in = nc.dram_tensor("fvinit", [T, 1], F32, kind="ExternalInput")
    score_out = nc.dram_tensor("score", [1, 1], F32, kind="ExternalOutput")
    path_out = nc.dram_tensor("path", [1, L], I32, kind="ExternalOutput")

    with tile.TileContext(nc) as tc:
        with tc.tile_pool(name="dram", bufs=1, space="DRAM") as dram:
            bp_dram = dram.tile([T, 8 * L], U16, name="bp_dram")
            bpT_rows = dram.tile([NB, 128 * T], F32, name="bpT_rows")
            with tc.tile_pool(name="const", bufs=1) as const:
                ident = const.tile([128, 128], F32, name="ident")
                make_identity(nc, ident[:])
                transT_sb = const.tile([T, T], F32, name="transT_sb")
                nc.sync.dma_start(transT_sb[:], transT_in[:])
                tstop_sb = const.tile([T, 1], F32, name="tstop_sb")
                nc.sync.dma_start(tstop_sb[:], tstop_in[:])
                bout_sb = const.tile([T, 1], F32, name="bout_sb")
                nc.sync.dma_start(bout_sb[:], bout_in[:])
                featT = const.tile([T, L], F32, name="featT")
                iota16 = const.tile([1, T], F32, name="iota16")
                nc.gpsimd.iota(iota16[:], pattern=[[1, T]], base=0,
                               channel_multiplier=0,
                               allow_small_or_imprecise_dtypes=True)
                fv = const.tile([T, 1], F32, name="fv")
                nc.sync.dma_start(fv[:], fvinit_in[:])
                scr32 = const.tile([32, 32], F32, name="scr32")
                nc.vector.memset(scr32[:], 0.0)
                scr32t = const.tile([32, 32], F32, name="scr32t")
                pathv = const.tile([1, L], F32, name="pathv")

                with tc.tile_pool(name="p3", bufs=3) as p3, \
                     tc.tile_pool(name="p3ps", bufs=2, space="PSUM") as p3ps:
                    for b in range(NB):
                        fb = p3.tile([128, T], F32, name="fb", tag="fb")
                        nc.sync.dma_start(fb[:], feats_in[ts(b, 128), :])
                        pf = p3ps.tile([T, 128], F32, name="pf", tag="pf")
                        nc.tensor.transpose(pf[:], fb[:], ident[:])
                        nc.vector.tensor_scalar(
                            featT[:, ts(b, 128)], pf[:], bout_sb[:], None,
                            OP.add)

                    # forward pass
                    for c in range(NB):
                        bp8 = p3.tile([T, 8 * 128], U16, name="bp8", tag="bp8")
                        mx8 = p3.tile([T, 8 * 128], F32, name="mx8", tag="mx8")
                        for tp in range(128):
                            t = c * 128 + tp
                            nc.vector.tensor_scalar(
                                scr32[0:T, 0:T], transT_sb[:], fv[:], None,
                                OP.add)
                            nc.vector.transpose(scr32t[:], scr32[:])
                            nc.vector.max(mx8[:, tp * 8:(tp + 1) * 8],
                                          scr32t[0:T, 0:T])
                            nc.vector.max_index(
                                bp8[:, tp * 8:(tp + 1) * 8],
                                mx8[:, tp * 8:(tp + 1) * 8],
                                scr32t[0:T, 0:T])
                            nc.vector.tensor_tensor(
                                out=fv[:], in0=mx8[:, tp * 8:tp * 8 + 1],
                                in1=featT[:, t:t + 1], op=OP.add)
                        nc.sync.dma_start(bp_dram[:, ts(c, 8 * 128)], bp8[:])

                    # terminal
                    term = p3.tile([T, 1], F32, name="term")
                    nc.vector.tensor_tensor(out=term[:], in0=fv[:],
                                            in1=tstop_sb[:], op=OP.add)
                    nc.vector.tensor_copy(scr32[0:T, 0:1], term[:])
                    nc.vector.transpose(scr32t[:], scr32[:])
                    tmx = p3.tile([1, 8], F32, name="tmx")
                    tix = p3.tile([1, 8], U16, name="tix")
                    nc.vector.max(tmx[:], scr32t[0:1, 0:T])
                    nc.vector.max_index(tix[:], tmx[:], scr32t[0:1, 0:T])
                    sc_sb = p3.tile([1, 1], F32, name="sc_sb")
                    nc.vector.tensor_copy(sc_sb[:], tmx[:, 0:1])
                    nc.sync.dma_start(score_out[:], sc_sb[:])
                    nc.vector.tensor_copy(pathv[:, L - 1:L], tix[:, 0:1])

                    # bp_dram -> bpT_rows
                    for c in range(NB):
                        bpc = p3.tile([T, 8 * 128], U16, name="bpc", tag="bpc")
                        nc.sync.dma_start(bpc[:], bp_dram[:, ts(c, 8 * 128)])
                        bpf = p3.tile([T, 128], F32, name="bpf", tag="bpf")
                        nc.vector.tensor_copy(bpf[:], bpc[:, 0:8 * 128:8])
                        pb = p3ps.tile([128, T], F32, name="pb", tag="pb")
                        nc.tensor.transpose(pb[:], bpf[:], ident[0:T, 0:T])
                        pbs = p3.tile([128, T], F32, name="pbs", tag="pbs")
                        nc.vector.tensor_copy(pbs[:], pb[:])
                        nc.sync.dma_start(
                            bpT_rows[c:c + 1, :].rearrange(
                                "p (a b) -> (p a) b", b=T),
                            pbs[:])

                    # backtrace
                    oh = const.tile([1, T], F32, name="oh")
                    dsc = const.tile([1, T], F32, name="dsc")
                    for c in range(NB - 1, -1, -1):
                        bch = p3.tile([1, 128 * T], F32, name="bch", tag="bch")
                        nc.sync.dma_start(bch[:], bpT_rows[c:c + 1, :])
                        for tp in range(127, -1, -1):
                            tn = c * 128 + tp
                            if tn == 0:
                                continue
                            nc.vector.tensor_scalar(
                                oh[:], iota16[:], pathv[:, tn:tn + 1], None,
                                OP.is_equal)
                            nc.vector.tensor_tensor(
                                out=dsc[:], in0=bch[:, tp * T:(tp + 1) * T],
                                in1=oh[:], op=OP.mult)
                            nc.vector.tensor_reduce(
                                out=pathv[:, tn - 1:tn], in_=dsc[:],
                                axis=mybir.AxisListType.X, op=OP.add)
                    pathi = const.tile([1, L], I32, name="pathi")
                    nc.vector.tensor_copy(pathi[:], pathv[:])
                    nc.sync.dma_start(path_out[:], pathi[:])
    nc.compile()
    return nc


# ----------------------------------------------------------------------------
# Host-side input preparation
# ----------------------------------------------------------------------------

def prep_gather_core(cfg: Cfg, core, sentence, emb):
    V, E, EP = cfg.V, cfg.E, cfg.EP
    sh = cfg.shard
    lo, hi = core * sh, min((core + 1) * sh, V)
    embshard = np.zeros((cfg.shard_rows, EP), np.float32)
    if lo < V:
        embshard[:hi - lo, :E] = emb[lo:hi]
    s = np.asarray(sentence).astype(np.int64)
    inrange = (s >= lo) & (s < hi)
    tok = np.where(inrange, s - lo, cfg.shard_rows - 1).astype(np.int32)
    tokfold = np.ascontiguousarray(tok.reshape(cfg.NB, 128).T)
    return dict(embshard=embshard, tokfold=tokfold)


def prep_lstm_core(cfg: Cfg, dirn, xsum, w_ih, w_hh, b, W_out, h0, c0):
    E, EP, NJ, NKT = cfg.E, cfg.EP, cfg.NJ, cfg.NKT
    H4 = 4 * H2
    perm = np.concatenate([np.arange(0, 2 * H2), np.arange(3 * H2, 4 * H2),
                           np.arange(2 * H2, 3 * H2)])
    xin = xsum if dirn == 0 else np.ascontiguousarray(xsum[::-1])

    w_ih_p = np.asarray(w_ih)[perm].astype(np.float32)
    wihT = np.zeros((128, len(cfg.KT) * H4), np.float32)
    for kt, ksz in enumerate(cfg.KT):
        w = min(ksz, E - kt * 128)
        wihT[:w, kt * H4:(kt + 1) * H4] = w_ih_p[:, kt * 128:kt * 128 + w].T
    bfold = np.ascontiguousarray(
        np.asarray(b)[perm].astype(np.float32).reshape(NJ, 128).T)

    w_hh_p = np.asarray(w_hh)[perm].astype(np.float32)
    whhpack = np.zeros((128, NJ * NKT * 128), np.float32)
    for j in range(NJ):
        for kt in range(NKT):
            blk = w_hh_p[j * 128:(j + 1) * 128, kt * 128:(kt + 1) * 128]
            whhpack[:, (j * NKT + kt) * 128:(j * NKT + kt + 1) * 128] = blk.T

    hc0fold = np.zeros((128, 4), np.float32)
    hv = np.asarray(h0)[dirn, 0].astype(np.float32)
    cv = np.asarray(c0)[dirn, 0].astype(np.float32)
    hc0fold[:, 0:2] = hv.reshape(NKT, 128).T
    hc0fold[:, 2:4] = cv.reshape(NKT, 128).T

    woT = np.zeros((128, NKT * T), np.float32)
    half = np.asarray(W_out)[:, dirn * H2:(dirn + 1) * H2].astype(np.float32)
    for kt in range(NKT):
        woT[:, kt * T:(kt + 1) * T] = half[:, kt * 128:(kt + 1) * 128].T

    return dict(xin=xin, wihT=wihT, bfold=bfold, whhpack=whhpack,
                hc0fold=hc0fold, woT=woT)


def prep_viterbi(cfg: Cfg, feats, transitions, b_out):
    tr = np.asarray(transitions).astype(np.float32)
    fvinit = np.full((T, 1), NEG, np.float32)
    fvinit[START, 0] = 0.0
    return dict(
        feats=np.ascontiguousarray(feats.astype(np.float32)),
        transT=np.ascontiguousarray(tr.T),
        tstopcol=np.ascontiguousarray(tr[STOP].reshape(T, 1)),
        boutcol=np.asarray(b_out).astype(np.float32).reshape(T, 1),
        fvinit=fvinit,
    )


_CACHED = {}


def get_ncs(cfg: Cfg):
    if cfg.key not in _CACHED:
        _CACHED[cfg.key] = (build_gather(cfg), build_lstm(cfg),
                            build_viterbi(cfg))
    return _CACHED[cfg.key]


def run(cfg: Cfg, inputs, trace=False):
    from concourse.bass_utils import run_bass_kernel_spmd
    nc_g, nc_l, nc_v = get_ncs(cfg)

    in_g = [prep_gather_core(cfg, c, inputs["sentence"], inputs["emb"])
            for c in range(cfg.n_gather)]
    res_g = run_bass_kernel_spmd(nc_g, in_g, list(range(cfg.n_gather)))
    xsum = res_g.results[0]["xpart"]
    for c in range(1, cfg.n_gather):
        xsum = xsum + res_g.results[c]["xpart"]

    in_l = [
        prep_lstm_core(cfg, 0, xsum, inputs["w_ih_f"], inputs["w_hh_f"],
                       inputs["b_f"], inputs["W_out"], inputs["h0"],
                       inputs["c0"]),
        prep_lstm_core(cfg, 1, xsum, inputs["w_ih_b"], inputs["w_hh_b"],
                       inputs["b_b"], inputs["W_out"], inputs["h0"],
                       inputs["c0"]),
    ]
    res_l = run_bass_kernel_spmd(nc_l, in_l, [0, 1])
    feats = res_l.results[0]["hproj"] + res_l.results[1]["hproj"][::-1]

    in_v = [prep_viterbi(cfg, feats, inputs["transitions"], inputs["b_out"])]
    res_v = run_bass_kernel_spmd(nc_v, in_v, [0], trace=trace)
    score = np.float32(res_v.results[0]["score"][0, 0])
    path = res_v.results[0]["path"][0].astype(np.int32)
    return score, path, (res_g, res_l, res_v)


def kernel(**inputs):
    cfg = Cfg()
    score, path, _ = run(cfg, inputs)
    return score, path
